# revision 11
# baseline (speedup 1.0000x reference)
"""Trainium2 Bass kernel for nn_AttnDecoderGRU.

Reference computation (per batch row b):
    cat1 = [last_input, h0]                       # [B, 576]
    attn_w = softmax(cat1 @ attn_W.T + attn_b)    # [B, 200]
    attn_applied = einsum('bs,bsh->bh', attn_w, enc)
    x = relu([last_input, attn_applied] @ comb_W.T + comb_b)
    gi = x @ w_ih.T + b_ih ; gh = h0 @ w_hh.T + b_hh
    r = sig(i_r+h_r); z = sig(i_z+h_z); n = tanh(i_n + r*h_n)
    h_new = (1-z)*n + z*h0 ; out = h_new @ out_W.T + out_b

Strategy: pure data-parallel over batch (4096 -> 8 cores x 512).
Weights are replicated. Activations on-device are kept in transposed
[feature, batch] layout so chained matmuls never need activation
transposes; the host pre-transposes the (tiny) weights and h0/last_input.

The memory-bound part is streaming encoder_outputs (cast to bf16 on the
host: 100 MB/core). The einsum runs on the TensorEngine as
PSUM-accumulated "diagonal matmuls": out_bt += diag(attn_w[bt,s]) @
enc[s,bt], with the diagonals built on the Vector/Scalar engines as
identity * attn_w[:, s] (per-partition scalar broadcast).

Pipelining: the batch is processed in two halves. Half 0's encoder
stream goes first; its entire post-attention pipeline (transpose, comb,
GRU, output) is interleaved into half 1's stream window, so only half
1's pipeline remains as serial tail. The h0-side GRU matmuls (gh, h_n),
which don't depend on attention, are likewise executed during the
stream. Host pre-swizzles enc so every DMA is 128 partitions x 8 KiB
contiguous.
"""

import sys
from contextlib import ExitStack

import ml_dtypes
import numpy as np

sys.path.insert(0, "/opt/trn_rl_repo")

import concourse.bass as bass  # noqa: E402
import concourse.tile as tile  # noqa: E402
from concourse import bacc, masks, mybir  # noqa: E402
from concourse._compat import with_exitstack  # noqa: E402
from concourse.bass_utils import run_bass_kernel_spmd  # noqa: E402

B, S, H, OUT = 4096, 200, 512, 64
NCORES = 8
BL = B // NCORES  # 512 rows of batch per core
NT = BL // 128    # 4 batch tiles per core
HT = H // 128     # 4 hidden-feature tiles
G3 = 3 * H        # 1536
CAT = H + OUT     # 576
NH = 2            # batch halves per core
BH = BL // NH     # 256 rows per half
SC = 4            # s-steps per enc DMA chunk (1 MiB bf16)

F32 = mybir.dt.float32
F32R = mybir.dt.float32r
BF16 = mybir.dt.bfloat16
AF = mybir.ActivationFunctionType
AX = mybir.AxisListType


@with_exitstack
def _body(ctx: ExitStack, tc: tile.TileContext, d: dict, s_len: int):
    nc = tc.nc

    const = ctx.enter_context(tc.tile_pool(name="const", bufs=1))
    encp = ctx.enter_context(tc.tile_pool(name="encp", bufs=6))
    diagp = ctx.enter_context(tc.tile_pool(name="diagp", bufs=4))
    big = ctx.enter_context(tc.tile_pool(name="big", bufs=1))
    scr = ctx.enter_context(tc.tile_pool(name="scr", bufs=3))
    small = ctx.enter_context(tc.tile_pool(name="small", bufs=4))
    psum = ctx.enter_context(tc.tile_pool(name="psum", bufs=3, space="PSUM"))
    pacc = ctx.enter_context(tc.tile_pool(name="pacc", bufs=1, space="PSUM"))

    # ---- load weights / small activations into SBUF ----
    # Critical-path tensors (scores need h0T/lastT/arhs) go on the SP HWDGE
    # ring first; the big GRU weights go via SWDGE so they don't delay the
    # enc stream on the SP ring.
    def load(name, shape, src, dt=F32, eng=None):
        t = const.tile(shape, dt, name=name, tag=name)
        (eng or nc.sync).dma_start(t[:], src)
        return t

    # folded layouts: [kt*128 + p, x] in DRAM -> sbuf [p, kt, x]
    h0T = load("h0T_sb", [128, HT, BL], d["h0T"].rearrange("(k p) b -> p k b", p=128), F32R)
    lastT = load("lastT_sb", [OUT + 1, BL], d["lastT"][:, :], F32R)
    arhs = load("arhs_sb", [128, 5, s_len], d["attn_rhs"].rearrange("(k p) s -> p k s", p=128), F32R)
    b_rz = load("b_rz_sb", [128, 8], d["b_rz"][:, :])
    b_ihn = load("b_ihn_sb", [128, HT], d["b_ihn"][:, :])
    b_hhn = load("b_hhn_sb", [128, HT], d["b_hhn"][:, :])
    comb_bT = load("comb_bT_sb", [128, HT], d["comb_bT"][:, :])
    out_b = load("out_b_sb", [OUT, 1], d["out_b"][:, :])
    gp = nc.gpsimd
    wih = load("wih_sb", [128, HT, G3], d["w_ihT"].rearrange("(k p) g -> p k g", p=128), F32R, gp)
    whh = load("whh_sb", [128, HT, G3], d["w_hhT"].rearrange("(k p) g -> p k g", p=128), F32R, gp)
    comb = load("comb_sb", [128, 5, H], d["comb_WT"].rearrange("(k p) h -> p k h", p=128), F32R, gp)
    owt = load("owt_sb", [128, HT, OUT], d["out_WT"].rearrange("(k p) o -> p k o", p=128), F32R, gp)

    ident = const.tile([128, 128], F32, name="ident")
    masks.make_identity(nc, ident[:])
    identb = const.tile([128, 128], BF16, name="identb")
    masks.make_identity(nc, identb[:])

    # ---- phase 1: attention scores + softmax -> attn_w (b-major) ----
    # scores[b, s] = sum_k cat1T[k, b] * attn_rhs[k, s]; cat1T k-order is
    # [h0 (512) ; last_input (64) ; ones (1)], bias folded via the ones row.
    attn_w = []
    for bt in range(NT):
        ps = psum.tile([128, s_len], F32, name=f"ps_scores{bt}", tag="ps")
        for kt in range(5):
            kk = 128 if kt < 4 else OUT + 1
            if kt < 4:
                lhsT = h0T[:, kt, bt * 128:(bt + 1) * 128]
            else:
                lhsT = lastT[0:kk, bt * 128:(bt + 1) * 128]
            rhs = arhs[0:kk, kt, :]
            nc.tensor.matmul(ps[:], lhsT, rhs, start=(kt == 0), stop=(kt == 4))
        negmax = small.tile([128, 1], F32, name=f"negmax{bt}", tag="negmax")
        ssum = small.tile([128, 1], F32, name=f"ssum{bt}", tag="ssum")
        rinv = small.tile([128, 1], F32, name=f"rinv{bt}", tag="rinv")
        nc.vector.reduce_max(negmax[:], ps[:], axis=AX.X, negate=True)
        aw = big.tile([128, s_len], F32, name=f"attn_w{bt}", tag=f"attn_w{bt}")
        nc.scalar.activation(aw[:], ps[:], AF.Exp, bias=negmax[:], accum_out=ssum[:])
        nc.vector.reciprocal(rinv[:], ssum[:])
        nc.vector.tensor_scalar_mul(aw[:], aw[:], rinv[:])
        nc.gpsimd.dma_start(d["attn_w"][bt * 128:(bt + 1) * 128, :], aw[:])
        attn_w.append(aw)

    # ---- gh/hn prefetch: h0-side GRU matmuls, run during the stream ----
    gh_sb = []
    hn_sb = []

    def gh_prefetch(mt):
        pgh = psum.tile([128, BL], F32, name=f"pgh{mt}", tag="ps")
        for kt in range(HT):
            lhsT = whh[:, kt, mt * 128:(mt + 1) * 128]
            nc.tensor.matmul(pgh[:], lhsT, h0T[:, kt, :],
                             start=(kt == 0), stop=(kt == HT - 1))
        if mt < 8:
            g = big.tile([128, BL], F32, name=f"gh_sb{mt}", tag=f"gh_sb{mt}")
            nc.scalar.copy(g[:], pgh[:])
            gh_sb.append(g)
        else:
            g = big.tile([128, BL], F32, name=f"hn_sb{mt}", tag=f"hn_sb{mt}")
            nc.scalar.add(g[:], pgh[:], b_hhn[:, mt - 8:mt - 7])
            hn_sb.append(g)

    gh_next = [0]

    def gh_step():
        if gh_next[0] < 12:
            gh_prefetch(gh_next[0])
            gh_next[0] += 1

    # ---- post-attention pipeline for one batch half, as a step list ----
    acc = [pacc.tile([128, H], F32, name=f"acc{t}", tag=f"acc{t}") for t in range(NT)]

    def post_steps(h):
        cols = slice(h * BH, (h + 1) * BH)
        steps = []
        aa, aaT, xT, rz, nn = [], [], [], [], []

        def s_aa(t2):
            a = big.tile([128, H], F32, name=f"aa{t2}", tag=f"aa{t2}")
            nc.scalar.copy(a[:], acc[h * NH + t2][:])
            aa.append(a)

        def s_tr(ht):
            at = big.tile([128, BH], F32R, name=f"aaT{ht}", tag=f"aaT{ht}")
            for u in range(NH):
                tp = psum.tile([128, 128], F32, name=f"tp{ht}_{u}", tag="ps")
                nc.tensor.transpose(tp[:], aa[u][:, ht * 128:(ht + 1) * 128], ident[:])
                nc.vector.tensor_copy(at[:, u * 128:(u + 1) * 128], tp[:])
            aaT.append(at)

        def s_comb(mt):
            px = psum.tile([128, BH], F32, name=f"px{mt}", tag="ps")
            for kt in range(5):
                kk = 128 if kt < 4 else OUT
                lhsT = comb[0:kk, kt, mt * 128:(mt + 1) * 128]
                rhs = aaT[kt][:] if kt < 4 else lastT[0:kk, cols]
                nc.tensor.matmul(px[:], lhsT, rhs, start=(kt == 0), stop=(kt == 4))
            xt = big.tile([128, BH], F32R, name=f"xT{mt}", tag=f"xT{mt}")
            nc.scalar.activation(xt[:], px[:], AF.Relu, bias=comb_bT[:, mt:mt + 1])
            xT.append(xt)

        def s_rz(mt):
            pg = psum.tile([128, BH], F32, name=f"pg{mt}", tag="ps")
            for kt in range(HT):
                lhsT = wih[:, kt, mt * 128:(mt + 1) * 128]
                nc.tensor.matmul(pg[:], lhsT, xT[kt][:],
                                 start=(kt == 0), stop=(kt == HT - 1))
            nc.vector.tensor_add(pg[:], pg[:], gh_sb[mt][:, cols])
            g = big.tile([128, BH], F32, name=f"rz{mt}", tag=f"rz{mt}")
            nc.scalar.activation(g[:], pg[:], AF.Sigmoid, bias=b_rz[:, mt:mt + 1])
            rz.append(g)

        def s_n(mt):
            pin = psum.tile([128, BH], F32, name=f"pin{mt}", tag="ps")
            for kt in range(HT):
                lhsT = wih[:, kt, (8 + mt) * 128:(9 + mt) * 128]
                nc.tensor.matmul(pin[:], lhsT, xT[kt][:],
                                 start=(kt == 0), stop=(kt == HT - 1))
            tmp = scr.tile([128, BH], F32, name=f"tmp{mt}", tag="tmp")
            nc.vector.tensor_mul(tmp[:], rz[mt][:], hn_sb[mt][:, cols])
            nc.vector.tensor_add(tmp[:], tmp[:], pin[:])
            n = big.tile([128, BH], F32R, name=f"n{mt}", tag=f"n{mt}")
            nc.scalar.activation(n[:], tmp[:], AF.Tanh, bias=b_ihn[:, mt:mt + 1])
            dlt = scr.tile([128, BH], F32, name=f"dlt{mt}", tag="dlt")
            nc.vector.tensor_sub(dlt[:], h0T[:, mt, cols], n[:])
            nc.vector.tensor_mul(dlt[:], rz[4 + mt][:], dlt[:])
            nc.vector.tensor_add(n[:], n[:], dlt[:])
            nc.gpsimd.dma_start(d["h_newT"][mt * 128:(mt + 1) * 128, cols], n[:])
            nn.append(n)

        def s_out():
            po = psum.tile([OUT, BH], F32, name="po", tag="ps")
            for kt in range(HT):
                nc.tensor.matmul(po[:], owt[:, kt, :], nn[kt][:],
                                 start=(kt == 0), stop=(kt == HT - 1))
            ot = big.tile([OUT, BH], F32, name="outT_sb", tag="outT_sb")
            nc.scalar.add(ot[:], po[:], out_b[:, 0:1])
            nc.gpsimd.dma_start(d["outT"][:, cols], ot[:])

        for t2 in range(NH):
            steps.append(lambda t2=t2: s_aa(t2))
        for ht in range(HT):
            steps.append(lambda ht=ht: s_tr(ht))
        for mt in range(HT):
            steps.append(lambda mt=mt: s_comb(mt))
        for mt in range(8):
            steps.append(lambda mt=mt: s_rz(mt))
        for mt in range(HT):
            steps.append(lambda mt=mt: s_n(mt))
        steps.append(s_out)
        return steps

    # ---- phase 2: stream enc halves; attn_applied accumulates in PSUM ----
    n_chunks = s_len // SC
    pending = []
    for h in range(NH):
        for ci in range(n_chunks):
            et = encp.tile([128, SC, NH, H], BF16, name=f"enc{h}_{ci}", tag="enc")
            nc.sync.dma_start(
                et[:],
                d["enc"][h, ci, :, :].rearrange("p (s t h) -> p s t h", s=SC, t=NH),
            )
            for j in range(SC):
                s = ci * SC + j
                for t2 in range(NH):
                    t = h * NH + t2
                    dg = diagp.tile([128, 128], BF16, name=f"dg{s}_{t}", tag="dg")
                    if t2 == 1 and j % 2 == 1:
                        nc.scalar.mul(dg[:], identb[:], attn_w[t][:, s:s + 1])
                    else:
                        nc.vector.tensor_scalar_mul(dg[:], identb[:], attn_w[t][:, s:s + 1])
                    nc.tensor.matmul(
                        acc[t][:], dg[:], et[:, j, t2, :],
                        start=(s == 0), stop=(s == s_len - 1),
                    )
            if h == 0 and ci % 4 == 1:
                gh_step()
            if h == 1 and pending:
                pending.pop(0)()
        if h == 0:
            while gh_next[0] < 12:
                gh_step()
            pending = post_steps(0)
    for st in pending:
        st()
    for st in post_steps(1):
        st()


def build(s_len=S):
    nc = bacc.Bacc("TRN2", target_bir_lowering=False, debug=False)
    d = {}

    def din(name, shape, dt=F32):
        d[name] = nc.dram_tensor(name, shape, dt, kind="ExternalInput").ap()

    def dout(name, shape, dt=F32):
        d[name] = nc.dram_tensor(name, shape, dt, kind="ExternalOutput").ap()

    din("enc", [NH, s_len // SC, 128, SC * NH * H], BF16)
    din("h0T", [H, BL], F32R)
    din("lastT", [OUT + 1, BL], F32R)
    din("attn_rhs", [640, s_len], F32R)
    din("comb_WT", [640, H], F32R)
    din("w_ihT", [H, G3], F32R)
    din("w_hhT", [H, G3], F32R)
    din("out_WT", [H, OUT], F32R)
    din("b_rz", [128, 8])
    din("b_ihn", [128, HT])
    din("b_hhn", [128, HT])
    din("comb_bT", [128, HT])
    din("out_b", [OUT, 1])
    dout("outT", [OUT, BL])
    dout("h_newT", [H, BL], F32R)
    dout("attn_w", [BL, s_len])

    with tile.TileContext(nc) as tc:
        _body(tc, d, s_len)
    nc.compile()
    return nc


def make_in_maps(inputs, s_len=S, ncores=NCORES):
    li = np.asarray(inputs["last_input"], np.float32)
    hid = np.asarray(inputs["hidden"], np.float32)[0]
    enc = np.asarray(inputs["encoder_outputs"], np.float32)
    attn_W = np.asarray(inputs["attn_W"], np.float32)
    attn_b = np.asarray(inputs["attn_b"], np.float32)
    comb_W = np.asarray(inputs["comb_W"], np.float32)
    comb_b = np.asarray(inputs["comb_b"], np.float32)
    w_ih = np.asarray(inputs["w_ih"], np.float32)
    w_hh = np.asarray(inputs["w_hh"], np.float32)
    b_ih = np.asarray(inputs["b_ih"], np.float32)
    b_hh = np.asarray(inputs["b_hh"], np.float32)
    out_W = np.asarray(inputs["out_W"], np.float32)
    out_b = np.asarray(inputs["out_b"], np.float32)

    nb = enc.shape[1]
    h0T = np.ascontiguousarray(hid.T)
    lastT = np.concatenate([li.T, np.ones((1, nb), np.float32)], axis=0)

    attn_rhs = np.zeros((640, s_len), np.float32)
    attn_rhs[0:H] = attn_W.T[OUT:CAT]     # h0 features
    attn_rhs[H:CAT] = attn_W.T[0:OUT]     # last_input features
    attn_rhs[CAT] = attn_b                # ones row -> bias
    comb_WT = np.zeros((640, H), np.float32)
    comb_WT[0:H] = comb_W.T[OUT:CAT]      # attn_applied features
    comb_WT[H:CAT] = comb_W.T[0:OUT]      # last_input features

    shared = {
        "attn_rhs": attn_rhs,
        "comb_WT": comb_WT,
        "w_ihT": np.ascontiguousarray(w_ih.T),
        "w_hhT": np.ascontiguousarray(w_hh.T),
        "out_WT": np.ascontiguousarray(out_W.T),
        "b_rz": np.ascontiguousarray((b_ih + b_hh)[:2 * H].reshape(8, 128).T),
        "b_ihn": np.ascontiguousarray(b_ih[2 * H:].reshape(HT, 128).T),
        "b_hhn": np.ascontiguousarray(b_hh[2 * H:].reshape(HT, 128).T),
        "comb_bT": np.ascontiguousarray(comb_b.reshape(HT, 128).T),
        "out_b": np.ascontiguousarray(out_b.reshape(OUT, 1)),
    }
    in_maps = []
    for c in range(ncores):
        sl = slice(c * BL, (c + 1) * BL)
        m = dict(shared)
        # enc swizzle: [S, BL, H] -> [half, S//SC, 128, SC*NH*H] so every
        # DMA chunk is 128 partitions x 8 KiB contiguous.
        ec = enc[:, sl, :].astype(ml_dtypes.bfloat16)
        ec = ec.reshape(s_len // SC, SC, NH, NH, 128, H)  # (ci, j, half, t2, p, h)
        ec = ec.transpose(2, 0, 4, 1, 3, 5)               # (half, ci, p, j, t2, h)
        m["enc"] = np.ascontiguousarray(ec).reshape(NH, s_len // SC, 128, SC * NH * H)
        m["h0T"] = np.ascontiguousarray(h0T[:, sl])
        m["lastT"] = np.ascontiguousarray(lastT[:, sl])
        in_maps.append(m)
    return in_maps


_NC_CACHE = {}


def kernel(trace=False, **inputs):
    if "full" not in _NC_CACHE:
        _NC_CACHE["full"] = build(S)
    nc = _NC_CACHE["full"]
    in_maps = make_in_maps(inputs)
    res = run_bass_kernel_spmd(nc, in_maps, core_ids=list(range(NCORES)), trace=trace)
    out = np.concatenate([r["outT"].T for r in res.results], axis=0)
    h_new = np.concatenate([r["h_newT"].T for r in res.results], axis=0)[None]
    attn_w = np.concatenate([r["attn_w"] for r in res.results], axis=0)
    kernel.last_results = res
    return out, h_new.astype(np.float32), attn_w


# revision 12
# speedup vs baseline: 1.2074x; 1.2074x over previous
"""Trainium2 Bass kernel for nn_AttnDecoderGRU.

Reference computation (per batch row b):
    cat1 = [last_input, h0]                       # [B, 576]
    attn_w = softmax(cat1 @ attn_W.T + attn_b)    # [B, 200]
    attn_applied = einsum('bs,bsh->bh', attn_w, enc)
    x = relu([last_input, attn_applied] @ comb_W.T + comb_b)
    gi = x @ w_ih.T + b_ih ; gh = h0 @ w_hh.T + b_hh
    r = sig(i_r+h_r); z = sig(i_z+h_z); n = tanh(i_n + r*h_n)
    h_new = (1-z)*n + z*h0 ; out = h_new @ out_W.T + out_b

Strategy: pure data-parallel over batch (4096 -> 8 cores x 512).
Weights are replicated. Activations on-device are kept in transposed
[feature, batch] layout so chained matmuls never need activation
transposes; the host pre-transposes the (tiny) weights and h0/last_input.

The memory-bound part is streaming encoder_outputs (cast to bf16 on the
host: 100 MB/core). The einsum runs on the TensorEngine as
PSUM-accumulated "diagonal matmuls": out_bt += diag(attn_w[bt,s]) @
enc[s,bt], with the diagonals built on the Vector/Scalar engines as
identity * attn_w[:, s] (per-partition scalar broadcast). Host
pre-swizzles enc so every 1 MiB DMA chunk is 128 partitions x 8 KiB
contiguous.

The h0-side GRU matmuls (gh, h_n), which don't depend on attention, are
executed during the stream window where the TensorEngine has slack.
The attention-dependent tail (transpose, comb, gi, output) uses bf16
operands to keep it short.
"""

import sys
from contextlib import ExitStack

import ml_dtypes
import numpy as np

sys.path.insert(0, "/opt/trn_rl_repo")

import concourse.bass as bass  # noqa: E402
import concourse.tile as tile  # noqa: E402
from concourse import bacc, masks, mybir  # noqa: E402
from concourse._compat import with_exitstack  # noqa: E402
from concourse.bass_utils import run_bass_kernel_spmd  # noqa: E402

B, S, H, OUT = 4096, 200, 512, 64
NCORES = 8
BL = B // NCORES  # 512 rows of batch per core
NT = BL // 128    # 4 batch tiles per core
HT = H // 128     # 4 hidden-feature tiles
G3 = 3 * H        # 1536
CAT = H + OUT     # 576
SC = 2            # s-steps per enc DMA chunk (1 MiB bf16)

F32 = mybir.dt.float32
F32R = mybir.dt.float32r
BF16 = mybir.dt.bfloat16
AF = mybir.ActivationFunctionType
AX = mybir.AxisListType


@with_exitstack
def _body(ctx: ExitStack, tc: tile.TileContext, d: dict, s_len: int):
    nc = tc.nc

    const = ctx.enter_context(tc.tile_pool(name="const", bufs=1))
    encp = ctx.enter_context(tc.tile_pool(name="encp", bufs=6))
    diagp = ctx.enter_context(tc.tile_pool(name="diagp", bufs=4))
    big = ctx.enter_context(tc.tile_pool(name="big", bufs=1))
    scr = ctx.enter_context(tc.tile_pool(name="scr", bufs=3))
    small = ctx.enter_context(tc.tile_pool(name="small", bufs=4))
    psum = ctx.enter_context(tc.tile_pool(name="psum", bufs=3, space="PSUM"))
    pacc = ctx.enter_context(tc.tile_pool(name="pacc", bufs=1, space="PSUM"))

    # ---- load weights / small activations into SBUF ----
    # Critical-path tensors (scores need h0T/lastT/arhs) go on the SP HWDGE
    # ring first; the big GRU weights go via SWDGE so they don't delay the
    # enc stream on the SP ring.
    def load(name, shape, src, dt=F32, eng=None):
        t = const.tile(shape, dt, name=name, tag=name)
        (eng or nc.sync).dma_start(t[:], src)
        return t

    # folded layouts: [kt*128 + p, x] in DRAM -> sbuf [p, kt, x]
    h0T = load("h0T_sb", [128, HT, BL], d["h0T"].rearrange("(k p) b -> p k b", p=128), F32R)
    lastT = load("lastT_sb", [OUT + 1, BL], d["lastT"][:, :], F32R)
    arhs = load("arhs_sb", [128, 5, s_len], d["attn_rhs"].rearrange("(k p) s -> p k s", p=128), F32R)
    b_rz = load("b_rz_sb", [128, 8], d["b_rz"][:, :])
    b_ihn = load("b_ihn_sb", [128, HT], d["b_ihn"][:, :])
    b_hhn = load("b_hhn_sb", [128, HT], d["b_hhn"][:, :])
    comb_bT = load("comb_bT_sb", [128, HT], d["comb_bT"][:, :])
    out_b = load("out_b_sb", [OUT, 1], d["out_b"][:, :])
    gp = nc.gpsimd
    lastTb = load("lastTb_sb", [OUT, BL], d["lastTb"][:, :], BF16, gp)
    wih = load("wih_sb", [128, HT, G3], d["w_ihT"].rearrange("(k p) g -> p k g", p=128), BF16, gp)
    whh = load("whh_sb", [128, HT, G3], d["w_hhT"].rearrange("(k p) g -> p k g", p=128), F32R, gp)
    comb = load("comb_sb", [128, 5, H], d["comb_WT"].rearrange("(k p) h -> p k h", p=128), BF16, gp)
    owt = load("owt_sb", [128, HT, OUT], d["out_WT"].rearrange("(k p) o -> p k o", p=128), F32R, gp)

    ident = const.tile([128, 128], F32, name="ident")
    masks.make_identity(nc, ident[:])
    identb = const.tile([128, 128], BF16, name="identb")
    masks.make_identity(nc, identb[:])

    # ---- phase 1: attention scores + softmax -> attn_w (b-major) ----
    # scores[b, s] = sum_k cat1T[k, b] * attn_rhs[k, s]; cat1T k-order is
    # [h0 (512) ; last_input (64) ; ones (1)], bias folded via the ones row.
    attn_w = []
    for bt in range(NT):
        ps = psum.tile([128, s_len], F32, name=f"ps_scores{bt}", tag="ps")
        for kt in range(5):
            kk = 128 if kt < 4 else OUT + 1
            if kt < 4:
                lhsT = h0T[:, kt, bt * 128:(bt + 1) * 128]
            else:
                lhsT = lastT[0:kk, bt * 128:(bt + 1) * 128]
            rhs = arhs[0:kk, kt, :]
            nc.tensor.matmul(ps[:], lhsT, rhs, start=(kt == 0), stop=(kt == 4))
        negmax = small.tile([128, 1], F32, name=f"negmax{bt}", tag="negmax")
        ssum = small.tile([128, 1], F32, name=f"ssum{bt}", tag="ssum")
        rinv = small.tile([128, 1], F32, name=f"rinv{bt}", tag="rinv")
        nc.vector.reduce_max(negmax[:], ps[:], axis=AX.X, negate=True)
        aw = big.tile([128, s_len], F32, name=f"attn_w{bt}", tag=f"attn_w{bt}")
        nc.scalar.activation(aw[:], ps[:], AF.Exp, bias=negmax[:], accum_out=ssum[:])
        nc.vector.reciprocal(rinv[:], ssum[:])
        nc.vector.tensor_scalar_mul(aw[:], aw[:], rinv[:])
        nc.gpsimd.dma_start(d["attn_w"][bt * 128:(bt + 1) * 128, :], aw[:])
        attn_w.append(aw)

    # ---- gh/hn prefetch: h0-side GRU matmuls, run during the stream ----
    gh_sb = []
    hn_sb = []

    def gh_prefetch(mt):
        pgh = psum.tile([128, BL], F32, name=f"pgh{mt}", tag="ps")
        for kt in range(HT):
            lhsT = whh[:, kt, mt * 128:(mt + 1) * 128]
            nc.tensor.matmul(pgh[:], lhsT, h0T[:, kt, :],
                             start=(kt == 0), stop=(kt == HT - 1))
        if mt < 8:
            g = big.tile([128, BL], F32, name=f"gh_sb{mt}", tag=f"gh_sb{mt}")
            nc.scalar.copy(g[:], pgh[:])
            gh_sb.append(g)
        else:
            g = big.tile([128, BL], F32, name=f"hn_sb{mt}", tag=f"hn_sb{mt}")
            nc.scalar.add(g[:], pgh[:], b_hhn[:, mt - 8:mt - 7])
            hn_sb.append(g)

    gh_next = [0]

    def gh_step():
        if gh_next[0] < 12:
            gh_prefetch(gh_next[0])
            gh_next[0] += 1

    # ---- phase 2: stream enc; attn_applied accumulates in PSUM ----
    acc = [pacc.tile([128, H], F32, name=f"acc{t}", tag=f"acc{t}") for t in range(NT)]
    for si in range(0, s_len, SC):
        et = encp.tile([128, SC, NT, H], BF16, name=f"enc{si}", tag="enc")
        nc.sync.dma_start(
            et[:],
            d["enc"][si // SC, :, :].rearrange("p (s t h) -> p s t h", s=SC, t=NT),
        )
        for j in range(SC):
            s = si + j
            for t in range(NT):
                dg = diagp.tile([128, 128], BF16, name=f"dg{s}_{t}", tag="dg")
                if t < 3:
                    nc.vector.tensor_scalar_mul(dg[:], identb[:], attn_w[t][:, s:s + 1])
                else:
                    nc.scalar.mul(dg[:], identb[:], attn_w[t][:, s:s + 1])
                nc.tensor.matmul(
                    acc[t][:], dg[:], et[:, j, t, :],
                    start=(s == 0), stop=(s == s_len - 1),
                )
        if (si // SC) % 8 == 4:
            gh_step()
    while gh_next[0] < 12:
        gh_step()

    # ---- phase 3: transpose attn_applied [b, h] -> aaT [h, b] ----
    aa = []
    for t in range(NT):
        a = big.tile([128, H], F32, name=f"aa{t}", tag=f"aa{t}")
        nc.scalar.copy(a[:], acc[t][:])
        aa.append(a)
    aaT = []
    for t in range(HT):
        at = big.tile([128, BL], BF16, name=f"aaT{t}", tag=f"aaT{t}")
        for u in range(NT):
            tp = psum.tile([128, 128], F32, name=f"tp{t}_{u}", tag="ps")
            nc.tensor.transpose(tp[:], aa[u][:, t * 128:(t + 1) * 128], ident[:])
            nc.vector.tensor_copy(at[:, u * 128:(u + 1) * 128], tp[:])
        aaT.append(at)

    # ---- phase 4: xT = relu(comb_WT.T @ cat2T + comb_b) ----
    # cat2T k-order: [attn_applied (512) ; last_input (64)]
    xT = []
    for mt in range(HT):
        px = psum.tile([128, BL], F32, name=f"px{mt}", tag="ps")
        for kt in range(5):
            kk = 128 if kt < 4 else OUT
            lhsT = comb[0:kk, kt, mt * 128:(mt + 1) * 128]
            rhs = aaT[kt][:] if kt < 4 else lastTb[0:kk, :]
            nc.tensor.matmul(px[:], lhsT, rhs, start=(kt == 0), stop=(kt == 4))
        xt = big.tile([128, BL], BF16, name=f"xT{mt}", tag=f"xT{mt}")
        nc.scalar.activation(xt[:], px[:], AF.Relu, bias=comb_bT[:, mt:mt + 1])
        xT.append(xt)

    # ---- phase 5: r/z gates: gi matmuls + parked gh, sigmoid ----
    rz = []
    for mt in range(8):
        pg = psum.tile([128, BL], F32, name=f"pg{mt}", tag="ps")
        for kt in range(HT):
            lhsT = wih[:, kt, mt * 128:(mt + 1) * 128]
            nc.tensor.matmul(pg[:], lhsT, xT[kt][:],
                             start=(kt == 0), stop=(kt == HT - 1))
        nc.vector.tensor_add(pg[:], pg[:], gh_sb[mt][:])
        g = big.tile([128, BL], F32, name=f"rz{mt}", tag=f"rz{mt}")
        nc.scalar.activation(g[:], pg[:], AF.Sigmoid, bias=b_rz[:, mt:mt + 1])
        rz.append(g)
    r, z = rz[:4], rz[4:]

    # ---- phase 6: n = tanh(i_n + r*h_n); h_new = n + z*(h0 - n) ----
    hnew = []
    for mt in range(HT):
        pin = psum.tile([128, BL], F32, name=f"pin{mt}", tag="ps")
        for kt in range(HT):
            lhsT = wih[:, kt, (8 + mt) * 128:(9 + mt) * 128]
            nc.tensor.matmul(pin[:], lhsT, xT[kt][:],
                             start=(kt == 0), stop=(kt == HT - 1))
        tmp = scr.tile([128, BL], F32, name=f"tmp{mt}", tag="tmp")
        nc.vector.tensor_mul(tmp[:], r[mt][:], hn_sb[mt][:])
        nc.vector.tensor_add(tmp[:], tmp[:], pin[:])
        n = big.tile([128, BL], F32R, name=f"n{mt}", tag=f"n{mt}")
        nc.scalar.activation(n[:], tmp[:], AF.Tanh, bias=b_ihn[:, mt:mt + 1])
        dlt = scr.tile([128, BL], F32, name=f"dlt{mt}", tag="dlt")
        nc.vector.tensor_sub(dlt[:], h0T[:, mt, :], n[:])
        nc.vector.tensor_mul(dlt[:], z[mt][:], dlt[:])
        nc.vector.tensor_add(n[:], n[:], dlt[:])
        nc.gpsimd.dma_start(d["h_newT"][mt * 128:(mt + 1) * 128, :], n[:])
        hnew.append(n)

    # ---- phase 7: outT = out_WT.T @ h_newT + out_b ----
    po = psum.tile([OUT, BL], F32, name="po", tag="ps")
    for kt in range(HT):
        nc.tensor.matmul(po[:], owt[:, kt, :], hnew[kt][:],
                         start=(kt == 0), stop=(kt == HT - 1))
    ot = big.tile([OUT, BL], F32, name="outT_sb", tag="outT_sb")
    nc.scalar.add(ot[:], po[:], out_b[:, 0:1])
    nc.gpsimd.dma_start(d["outT"][:, :], ot[:])


def build(s_len=S):
    nc = bacc.Bacc("TRN2", target_bir_lowering=False, debug=False)
    d = {}

    def din(name, shape, dt=F32):
        d[name] = nc.dram_tensor(name, shape, dt, kind="ExternalInput").ap()

    def dout(name, shape, dt=F32):
        d[name] = nc.dram_tensor(name, shape, dt, kind="ExternalOutput").ap()

    din("enc", [s_len // SC, 128, SC * NT * H], BF16)
    din("h0T", [H, BL], F32R)
    din("lastT", [OUT + 1, BL], F32R)
    din("lastTb", [OUT, BL], BF16)
    din("attn_rhs", [640, s_len], F32R)
    din("comb_WT", [640, H], BF16)
    din("w_ihT", [H, G3], BF16)
    din("w_hhT", [H, G3], F32R)
    din("out_WT", [H, OUT], F32R)
    din("b_rz", [128, 8])
    din("b_ihn", [128, HT])
    din("b_hhn", [128, HT])
    din("comb_bT", [128, HT])
    din("out_b", [OUT, 1])
    dout("outT", [OUT, BL])
    dout("h_newT", [H, BL], F32R)
    dout("attn_w", [BL, s_len])

    with tile.TileContext(nc) as tc:
        _body(tc, d, s_len)
    nc.compile()
    return nc


def make_in_maps(inputs, s_len=S, ncores=NCORES):
    li = np.asarray(inputs["last_input"], np.float32)
    hid = np.asarray(inputs["hidden"], np.float32)[0]
    enc = np.asarray(inputs["encoder_outputs"], np.float32)
    attn_W = np.asarray(inputs["attn_W"], np.float32)
    attn_b = np.asarray(inputs["attn_b"], np.float32)
    comb_W = np.asarray(inputs["comb_W"], np.float32)
    comb_b = np.asarray(inputs["comb_b"], np.float32)
    w_ih = np.asarray(inputs["w_ih"], np.float32)
    w_hh = np.asarray(inputs["w_hh"], np.float32)
    b_ih = np.asarray(inputs["b_ih"], np.float32)
    b_hh = np.asarray(inputs["b_hh"], np.float32)
    out_W = np.asarray(inputs["out_W"], np.float32)
    out_b = np.asarray(inputs["out_b"], np.float32)

    nb = enc.shape[1]
    h0T = np.ascontiguousarray(hid.T)
    lastT = np.concatenate([li.T, np.ones((1, nb), np.float32)], axis=0)

    attn_rhs = np.zeros((640, s_len), np.float32)
    attn_rhs[0:H] = attn_W.T[OUT:CAT]     # h0 features
    attn_rhs[H:CAT] = attn_W.T[0:OUT]     # last_input features
    attn_rhs[CAT] = attn_b                # ones row -> bias
    comb_WT = np.zeros((640, H), np.float32)
    comb_WT[0:H] = comb_W.T[OUT:CAT]      # attn_applied features
    comb_WT[H:CAT] = comb_W.T[0:OUT]      # last_input features

    shared = {
        "attn_rhs": attn_rhs,
        "comb_WT": comb_WT.astype(ml_dtypes.bfloat16),
        "w_ihT": np.ascontiguousarray(w_ih.T).astype(ml_dtypes.bfloat16),
        "w_hhT": np.ascontiguousarray(w_hh.T),
        "out_WT": np.ascontiguousarray(out_W.T),
        "b_rz": np.ascontiguousarray((b_ih + b_hh)[:2 * H].reshape(8, 128).T),
        "b_ihn": np.ascontiguousarray(b_ih[2 * H:].reshape(HT, 128).T),
        "b_hhn": np.ascontiguousarray(b_hh[2 * H:].reshape(HT, 128).T),
        "comb_bT": np.ascontiguousarray(comb_b.reshape(HT, 128).T),
        "out_b": np.ascontiguousarray(out_b.reshape(OUT, 1)),
    }
    in_maps = []
    for c in range(ncores):
        sl = slice(c * BL, (c + 1) * BL)
        m = dict(shared)
        # enc swizzle: [S, BL, H] -> [S//SC, 128, SC*NT*H] so every DMA
        # chunk is 128 partitions x 8 KiB contiguous.
        ec = enc[:, sl, :].astype(ml_dtypes.bfloat16)
        ec = ec.reshape(s_len // SC, SC, NT, 128, H)  # (ci, j, t, p, h)
        ec = ec.transpose(0, 3, 1, 2, 4)              # (ci, p, j, t, h)
        m["enc"] = np.ascontiguousarray(ec).reshape(s_len // SC, 128, SC * NT * H)
        m["h0T"] = np.ascontiguousarray(h0T[:, sl])
        m["lastT"] = np.ascontiguousarray(lastT[:, sl])
        m["lastTb"] = np.ascontiguousarray(li.T[:, sl]).astype(ml_dtypes.bfloat16)
        in_maps.append(m)
    return in_maps


_NC_CACHE = {}


def kernel(trace=False, **inputs):
    if "full" not in _NC_CACHE:
        _NC_CACHE["full"] = build(S)
    nc = _NC_CACHE["full"]
    in_maps = make_in_maps(inputs)
    res = run_bass_kernel_spmd(nc, in_maps, core_ids=list(range(NCORES)), trace=trace)
    out = np.concatenate([r["outT"].T for r in res.results], axis=0)
    h_new = np.concatenate([r["h_newT"].T for r in res.results], axis=0)[None]
    attn_w = np.concatenate([r["attn_w"] for r in res.results], axis=0)
    kernel.last_results = res
    return out, h_new.astype(np.float32), attn_w
